# revision 1
# baseline (speedup 1.0000x reference)
"""Trainium2 Bass kernel for AttentionWithGeGLU pooling.

Math (per batch row b):
  q[s]   = sum_d x[b,s,d]^2
  rs[s]  = (q/D + eps)^-1/2
  t[s]   = sum_d x[b,s,d] * (ln_w*att_w)[d]
  score  = rs * t            (att_b dropped: softmax is shift-invariant)
  e      = exp(score);  denom = sum_s e
  pooled[b,d] = ln_w[d]/denom * sum_s (e[s]*rs[s]) * x[b,s,d]
  h      = pooled @ geglu_w + geglu_b;  out = val * gelu(gate)

Sharding: two NEFF launches (collectives are unavailable on this runtime):
  A) data-parallel pooling over batch (4 batches/core) -> pooled [4,1024]/core
  B) tensor-parallel GeGLU: host gathers+transposes pooled (128 KB), each
     core computes its 512 matching val+gate columns.
"""

import os
import numpy as np

B, S, D, OUT = 32, 2048, 1024, 4096
EPS = 1e-6
NCORES = 8
NB = B // NCORES          # batches per core
COLS = OUT // NCORES      # val columns per core
P = 128
NT = S // P               # seq tiles per batch

_cache = {}


def _build_nc_pool(mm="xbf16", dve_q_every=8):
    """Pooling NEFF. mm="xbf16": x arrives host-converted to bf16 (halves
    HBM traffic); q/t/pooled computed from bf16 x with fp32 accumulation.
    Every `dve_q_every`-th tile computes q on DVE instead of ACT to balance
    the two engines."""
    import concourse.bacc as bacc
    import concourse.mybir as mybir
    import concourse.tile as tile
    from contextlib import ExitStack

    f32 = mybir.dt.float32
    bf16 = mybir.dt.bfloat16
    xdt = bf16 if mm == "xbf16" else f32
    AF = mybir.ActivationFunctionType
    OP = mybir.AluOpType
    AX = mybir.AxisListType

    nc = bacc.Bacc(
        "TRN2",
        target_bir_lowering=False,
        debug=False,
        enable_asserts=False,
        num_devices=NCORES,
    )

    GRP = 4          # tiles per softmax/matmul group; one DMA per group
    NG = NT // GRP   # groups per batch

    x_d = nc.dram_tensor("x", [NB, S, D], xdt, kind="ExternalInput").ap()
    a_d = nc.dram_tensor("a", [1, D], xdt, kind="ExternalInput").ap()
    lnw_d = nc.dram_tensor("lnw", [1, D], f32, kind="ExternalInput").ap()
    cst_d = nc.dram_tensor("cst", [1, 2], f32, kind="ExternalInput").ap()
    pooled_d = nc.dram_tensor("pooled", [NB, D], f32, kind="ExternalOutput").ap()

    with tile.TileContext(nc) as tc, ExitStack() as ctx:
        singles = ctx.enter_context(tc.tile_pool(name="singles", bufs=1))
        xpool = ctx.enter_context(tc.tile_pool(name="xp", bufs=7))
        scratch = ctx.enter_context(tc.tile_pool(name="scr", bufs=2))
        small = ctx.enter_context(tc.tile_pool(name="small", bufs=3))
        psum_pool = ctx.enter_context(
            tc.tile_pool(name="pspool", bufs=2, space="PSUM")
        )
        psum_small = ctx.enter_context(
            tc.tile_pool(name="pssm", bufs=2, space="PSUM")
        )

        if os.environ.get("KERNEL_TABLELOAD", "0") == "1":
            # Preload the one act-table set containing Square+Ln+Exp so the
            # table-load fixpoint doesn't thrash between per-func sets.
            from concourse.hw_specs import get_activation_tables
            _tables = get_activation_tables(nc.m.arch)
            _set_id = list(_tables).index("natural_log_exp_and_others")
            _ld = mybir.InstLoadActFuncSet(
                name=nc.get_next_instruction_name(), ins=[], outs=[],
                act_func_set_id=_set_id,
            )
            nc.scalar.add_instruction(_ld)

        a_bc = singles.tile([P, D], xdt)
        nc.sync.dma_start(out=a_bc, in_=a_d.to_broadcast([P, D]))
        lnw_sb = singles.tile([1, D], f32)
        nc.sync.dma_start(out=lnw_sb, in_=lnw_d)
        # constants via DMA broadcast (DVE memset is unreliable on this runtime)
        ones = singles.tile([P, 1], f32)
        nc.sync.dma_start(out=ones, in_=cst_d[0:1, 0:1].to_broadcast([P, 1]))
        eps_col = singles.tile([P, 1], f32)
        nc.sync.dma_start(out=eps_col, in_=cst_d[0:1, 1:2].to_broadcast([P, 1]))

        pooled_sb = singles.tile([1, NB, D], f32)

        for b in range(NB):
            q_all = small.tile([P, NT], f32, tag="q")
            t_all = small.tile([P, NT], f32, tag="t")
            e_all = small.tile([P, NT], f32, tag="e")
            pp = psum_pool.tile([1, D], f32, tag="acc")
            for g in range(NG):
                xt = xpool.tile([P, GRP, D], xdt, tag="x")
                if os.environ.get("KERNEL_GRPDMA", "0") == "1":
                    nc.sync.dma_start(
                        out=xt,
                        in_=x_d[b, g * GRP * P:(g + 1) * GRP * P, :].rearrange(
                            "(grp p) d -> p grp d", p=P
                        ),
                    )
                else:
                    for jj in range(GRP):
                        j = g * GRP + jj
                        nc.sync.dma_start(
                            out=xt[:, jj, :],
                            in_=x_d[b, j * P:(j + 1) * P, :],
                        )
                for jj in range(GRP):
                    j = g * GRP + jj
                    # q: ACT square (plain), then DVE row-reduce.
                    # The accum_out fast path is NRT-fatal on this runtime.
                    sq = scratch.tile([P, D], xdt, tag="sq")
                    nc.scalar.activation(out=sq, in_=xt[:, jj, :],
                                         func=AF.Square)
                    nc.vector.reduce_sum(q_all[:, j:j + 1], sq, axis=AX.X)
                    tp = scratch.tile([P, D], xdt, tag="tp")
                    nc.vector.tensor_mul(tp, xt[:, jj, :], a_bc)
                    nc.vector.reduce_sum(t_all[:, j:j + 1], tp, axis=AX.X)

                gs = slice(g * GRP, (g + 1) * GRP)
                # rs = (q/D + eps)^-1/2 via fast-inverse-sqrt + 3 Newton
                # steps on DVE (avoids Ln/Exp table traffic; Exp for the
                # softmax is then the only other ACT function in use and
                # shares Square's table set).
                v = small.tile([P, GRP], f32, tag="v")
                nc.vector.tensor_scalar(
                    out=v, in0=q_all[:, gs], scalar1=1.0 / D, scalar2=EPS,
                    op0=OP.mult, op1=OP.add)
                # v = mean(x^2)+eps is ~1 for unit-variance rows, so Newton
                # from the first iterate y1 = 1.5 - 0.5*v converges fast.
                y = small.tile([P, GRP], f32, tag="y")
                nc.vector.tensor_scalar(
                    out=y, in0=v, scalar1=-0.5, scalar2=1.5,
                    op0=OP.mult, op1=OP.add)
                for _ in range(3):
                    u = small.tile([P, GRP], f32, tag="u")
                    nc.vector.tensor_mul(u, y, y)
                    nc.vector.tensor_mul(u, u, v)
                    nc.vector.tensor_scalar(
                        out=u, in0=u, scalar1=-0.5, scalar2=1.5,
                        op0=OP.mult, op1=OP.add)
                    nc.vector.tensor_mul(y, y, u)
                rs = y
                sc = small.tile([P, GRP], f32, tag="sc")
                nc.vector.tensor_mul(sc, t_all[:, gs], rs)
                nc.scalar.activation(out=e_all[:, gs], in_=sc, func=AF.Exp)
                c_g = small.tile([P, GRP], xdt, tag="c")
                nc.vector.tensor_mul(c_g, e_all[:, gs], rs)

                # pass B for this group: pooled_raw[1, D] += c_j^T @ x_j
                for jj in range(GRP):
                    for h in range(2):
                        nc.tensor.matmul(
                            pp[0:1, h * 512:(h + 1) * 512],
                            lhsT=c_g[:, jj:jj + 1],
                            rhs=xt[:, jj, h * 512:(h + 1) * 512],
                            start=(g == 0 and jj == 0),
                            stop=(g == NG - 1 and jj == GRP - 1),
                        )

            # denom = sum of e over all s
            dps = psum_small.tile([1, NT], f32, tag="sm")
            nc.tensor.matmul(dps, lhsT=ones, rhs=e_all, start=True, stop=True)
            dsum = small.tile([1, 1], f32, tag="dsum")
            nc.vector.reduce_sum(dsum, dps, axis=AX.X)
            invd = small.tile([1, 1], f32, tag="invd")
            nc.vector.reciprocal(invd, dsum)
            # pooled = pooled_raw * invd * ln_w
            nc.vector.scalar_tensor_tensor(
                out=pooled_sb[0:1, b, :], in0=pp[0:1, :], scalar=invd,
                in1=lnw_sb, op0=OP.mult, op1=OP.mult,
            )

        for b in range(NB):
            nc.sync.dma_start(out=pooled_d[b:b + 1, :],
                              in_=pooled_sb[0:1, b, :])

    nc.compile()
    return nc




def _build_nc_pool_classic():
    """Conservative pool NEFF: fp32 x, per-tile DMAs, per-batch softmax,
    fp32 matmuls — mirrors the structure already proven to execute on HW."""
    import concourse.bacc as bacc
    import concourse.mybir as mybir
    import concourse.tile as tile
    from contextlib import ExitStack

    f32 = mybir.dt.float32
    AF = mybir.ActivationFunctionType
    OP = mybir.AluOpType
    AX = mybir.AxisListType

    nc = bacc.Bacc("TRN2", target_bir_lowering=False, debug=False,
                   enable_asserts=False, num_devices=NCORES)

    x_d = nc.dram_tensor("x", [NB, S, D], f32, kind="ExternalInput").ap()
    a_d = nc.dram_tensor("a", [1, D], f32, kind="ExternalInput").ap()
    lnw_d = nc.dram_tensor("lnw", [1, D], f32, kind="ExternalInput").ap()
    cst_d = nc.dram_tensor("cst", [1, 2], f32, kind="ExternalInput").ap()
    pooled_d = nc.dram_tensor("pooled", [NB, D], f32, kind="ExternalOutput").ap()

    with tile.TileContext(nc) as tc, ExitStack() as ctx:
        singles = ctx.enter_context(tc.tile_pool(name="singles", bufs=1))
        xpool = ctx.enter_context(tc.tile_pool(name="xp", bufs=26))
        scratch = ctx.enter_context(tc.tile_pool(name="scr", bufs=2))
        small = ctx.enter_context(tc.tile_pool(name="small", bufs=3))
        psum_pool = ctx.enter_context(tc.tile_pool(name="pspool", bufs=2, space="PSUM"))
        psum_small = ctx.enter_context(tc.tile_pool(name="pssm", bufs=2, space="PSUM"))

        a_bc = singles.tile([P, D], f32)
        nc.sync.dma_start(out=a_bc, in_=a_d.to_broadcast([P, D]))
        lnw_sb = singles.tile([1, D], f32)
        nc.sync.dma_start(out=lnw_sb, in_=lnw_d)
        # constants via DMA broadcast (DVE memset is unreliable on this runtime)
        ones = singles.tile([P, 1], f32)
        nc.sync.dma_start(out=ones, in_=cst_d[0:1, 0:1].to_broadcast([P, 1]))
        eps_col = singles.tile([P, 1], f32)
        nc.sync.dma_start(out=eps_col, in_=cst_d[0:1, 1:2].to_broadcast([P, 1]))

        pooled_sb = singles.tile([1, NB, D], f32)

        for b in range(NB):
            q_all = small.tile([P, NT], f32, tag="q")
            t_all = small.tile([P, NT], f32, tag="t")
            x_tiles = []
            for j in range(NT):
                xt = xpool.tile([P, D], f32, tag="x")
                nc.sync.dma_start(out=xt, in_=x_d[b, j * P:(j + 1) * P, :])
                x_tiles.append(xt)
                sq = scratch.tile([P, D], f32, tag="sq")
                nc.scalar.activation(out=sq, in_=xt, func=AF.Square)
                nc.vector.reduce_sum(q_all[:, j:j + 1], sq, axis=AX.X)
                tp = scratch.tile([P, D], f32, tag="tp")
                nc.vector.tensor_mul(tp, xt, a_bc)
                nc.vector.reduce_sum(t_all[:, j:j + 1], tp, axis=AX.X)

            # rs = 1/sqrt(q/D + eps)  (groupnorm's sqrt+reciprocal recipe)
            rs = small.tile([P, NT], f32, tag="rs")
            nc.scalar.activation(out=rs, in_=q_all, func=AF.Sqrt,
                                 scale=1.0 / D, bias=eps_col)
            nc.vector.reciprocal(rs, rs)
            sc = small.tile([P, NT], f32, tag="sc")
            nc.vector.tensor_mul(sc, t_all, rs)
            e_all = small.tile([P, NT], f32, tag="e")
            nc.scalar.activation(out=e_all, in_=sc, func=AF.Exp)
            c_all = small.tile([P, NT], f32, tag="c")
            nc.vector.tensor_mul(c_all, e_all, rs)

            dps = psum_small.tile([1, NT], f32, tag="sm")
            nc.tensor.matmul(dps, lhsT=ones, rhs=e_all, start=True, stop=True)
            dsum = small.tile([1, 1], f32, tag="dsum")
            nc.vector.reduce_sum(dsum, dps, axis=AX.X)
            invd = small.tile([1, 1], f32, tag="invd")
            nc.vector.reciprocal(invd, dsum)

            pp = psum_pool.tile([1, D], f32, tag="acc")
            for j in range(NT):
                for h in range(2):
                    nc.tensor.matmul(
                        pp[0:1, h * 512:(h + 1) * 512],
                        lhsT=c_all[:, j:j + 1],
                        rhs=x_tiles[j][:, h * 512:(h + 1) * 512],
                        start=(j == 0), stop=(j == NT - 1))
            nc.vector.scalar_tensor_tensor(
                out=pooled_sb[0:1, b, :], in0=pp[0:1, :], scalar=invd,
                in1=lnw_sb, op0=OP.mult, op1=OP.mult)

        for b in range(NB):
            nc.sync.dma_start(out=pooled_d[b:b + 1, :],
                              in_=pooled_sb[0:1, b, :])

    nc.compile()
    return nc

def _build_nc_geglu(mm="bf16x2"):
    import concourse.bacc as bacc
    import concourse.mybir as mybir
    import concourse.tile as tile
    from contextlib import ExitStack

    f32 = mybir.dt.float32
    bf16 = mybir.dt.bfloat16
    comp = mm == "bf16x2"   # compensated bf16: hi/lo split of both operands
    mdt = f32 if mm == "fp32" else bf16
    NIN = 2 if comp else 1
    AF = mybir.ActivationFunctionType

    nc = bacc.Bacc(
        "TRN2",
        target_bir_lowering=False,
        debug=False,
        enable_asserts=False,
        num_devices=NCORES,
    )

    pT_d = nc.dram_tensor("pT", [P, NIN, 8, B], mdt, kind="ExternalInput").ap()
    w_d = nc.dram_tensor("w", [NIN, 8, P, 2 * COLS], mdt, kind="ExternalInput").ap()
    bias_d = nc.dram_tensor("bias", [1, 2 * COLS], f32, kind="ExternalInput").ap()
    out_d = nc.dram_tensor("out", [B, COLS], f32, kind="ExternalOutput").ap()

    with tile.TileContext(nc) as tc, ExitStack() as ctx:
        singles = ctx.enter_context(tc.tile_pool(name="singles", bufs=1))
        tailp = ctx.enter_context(tc.tile_pool(name="tail", bufs=2))
        psum_pool = ctx.enter_context(
            tc.tile_pool(name="pspool", bufs=1, space="PSUM")
        )

        pT_sb = singles.tile([P, NIN, 8, B], mdt)
        nc.sync.dma_start(out=pT_sb, in_=pT_d)
        # per-chunk DMAs so matmul k can start as soon as chunk k lands;
        # all hi chunks stream before the lo chunks
        w_sb = singles.tile([P, NIN, 8, 2 * COLS], mdt)
        for n in range(NIN):
            for k in range(8):
                nc.sync.dma_start(out=w_sb[:, n, k], in_=w_d[n, k])
        bias_bc = singles.tile([B, 2 * COLS], f32)
        nc.sync.dma_start(out=bias_bc, in_=bias_d.to_broadcast([B, 2 * COLS]))

        # terms: hi@hi (+ lo@hi + hi@lo when compensated); the w_lo term
        # goes last since the lo half of W streams in after the hi half
        terms = [(0, 0)] if not comp else [(0, 0), (1, 0), (0, 1)]
        hps = psum_pool.tile([B, 2 * COLS], f32, tag="acc")
        for ti, (pn, wn) in enumerate(terms):
            for k in range(8):
                for h in range(2):
                    nc.tensor.matmul(
                        hps[:, h * COLS:(h + 1) * COLS],
                        lhsT=pT_sb[:, pn, k, :],
                        rhs=w_sb[:, wn, k, h * COLS:(h + 1) * COLS],
                        start=(ti == 0 and k == 0),
                        stop=(ti == len(terms) - 1 and k == 7),
                    )
        hv = tailp.tile([B, COLS], f32, tag="hv")
        nc.vector.tensor_add(hv, hps[:, 0:COLS], bias_bc[:, 0:COLS])
        hg = tailp.tile([B, COLS], f32, tag="hg")
        nc.vector.tensor_add(hg, hps[:, COLS:2 * COLS], bias_bc[:, COLS:2 * COLS])
        gg = tailp.tile([B, COLS], f32, tag="gg")
        nc.scalar.activation(out=gg, in_=hg, func=AF.Gelu)
        outt = tailp.tile([B, COLS], f32, tag="outt")
        nc.vector.tensor_mul(outt, hv, gg)
        nc.sync.dma_start(out=out_d, in_=outt)

    nc.compile()
    return nc


def _pool_in_maps(x, ln_w, att_w, mm="xbf16"):
    import ml_dtypes
    xdt = ml_dtypes.bfloat16 if mm == "xbf16" else np.float32
    if mm == "classic":
        xdt = np.float32
    a = (ln_w * att_w[:, 0]).astype(xdt).reshape(1, D)
    lnw = ln_w.astype(np.float32).reshape(1, D)
    xc = np.ascontiguousarray(x.astype(xdt))
    cst = np.array([[1.0, EPS]], dtype=np.float32)
    return [
        {"x": xc[r * NB:(r + 1) * NB], "a": a, "lnw": lnw, "cst": cst}
        for r in range(NCORES)
    ]


def _split_hi_lo(arr, comp):
    import ml_dtypes
    if not comp:
        return arr.astype(ml_dtypes.bfloat16)[None]
    hi = arr.astype(ml_dtypes.bfloat16)
    lo = (arr - hi.astype(np.float32)).astype(ml_dtypes.bfloat16)
    return np.stack([hi, lo])


def _geglu_in_maps(pooled_full, geglu_w, geglu_b, mm="bf16x2"):
    comp = mm == "bf16x2"
    NIN = 2 if comp else 1
    if mm == "fp32":
        def conv(a):
            return a.astype(np.float32)[None]
    else:
        def conv(a):
            return _split_hi_lo(a, comp)
    pTn = np.ascontiguousarray(
        conv(np.ascontiguousarray(pooled_full.T))
    ).reshape(NIN, 8, P, B)
    pT = np.ascontiguousarray(np.transpose(pTn, (2, 0, 1, 3)))
    maps = []
    for r in range(NCORES):
        vs = slice(r * COLS, (r + 1) * COLS)
        gs = slice(OUT + r * COLS, OUT + (r + 1) * COLS)
        wcat = np.ascontiguousarray(
            np.concatenate([geglu_w[:, vs], geglu_w[:, gs]], axis=1)
        )
        wr = np.ascontiguousarray(conv(wcat)).reshape(NIN, 8, P, 2 * COLS)
        br = np.ascontiguousarray(
            np.concatenate([geglu_b[vs], geglu_b[gs]])
        ).reshape(1, 2 * COLS)
        maps.append({"pT": pT, "w": wr, "bias": br})
    return maps


LAST_RESULTS = None


def kernel(x, ln_w, att_w, att_b, geglu_w, geglu_b):
    global LAST_RESULTS
    from concourse.bass_utils import run_bass_kernel_spmd

    x = np.asarray(x, dtype=np.float32)
    ln_w = np.asarray(ln_w, dtype=np.float32)
    att_w = np.asarray(att_w, dtype=np.float32)
    geglu_w = np.asarray(geglu_w, dtype=np.float32)
    geglu_b = np.asarray(geglu_b, dtype=np.float32)
    # att_b is mathematically irrelevant (softmax shift-invariance)

    mm = os.environ.get("KERNEL_MM", "classic")
    gg = os.environ.get("KERNEL_GG", "fp32")
    if ("A", mm) not in _cache:
        if mm == "classic":
            _cache[("A", mm)] = _build_nc_pool_classic()
        else:
            _cache[("A", mm)] = _build_nc_pool(mm=mm)
    if ("B", gg) not in _cache:
        _cache[("B", gg)] = _build_nc_geglu(mm=gg)

    trace = os.environ.get("KERNEL_TRACE", "0") == "1"

    res_a = run_bass_kernel_spmd(
        _cache[("A", mm)], _pool_in_maps(x, ln_w, att_w, mm=mm),
        core_ids=list(range(NCORES)), trace=trace,
    )
    pooled_full = np.concatenate(
        [res_a.results[r]["pooled"] for r in range(NCORES)], axis=0
    )
    res_b = run_bass_kernel_spmd(
        _cache[("B", gg)], _geglu_in_maps(pooled_full, geglu_w, geglu_b, mm=gg),
        core_ids=list(range(NCORES)), trace=trace,
    )
    LAST_RESULTS = (res_a, res_b)
    out = np.concatenate(
        [res_b.results[r]["out"] for r in range(NCORES)], axis=1
    )
    return out.astype(np.float32)



# revision 6
# speedup vs baseline: 2.2344x; 2.2344x over previous
"""Trainium2 Bass kernel for AttentionWithGeGLU pooling.

Math (per batch row b):
  q[s]   = sum_d x[b,s,d]^2
  rs[s]  = (q/D + eps)^-1/2          (1 Newton step from y0 = 1.5 - 0.5 v)
  t[s]   = sum_d x[b,s,d] * (ln_w*att_w)[d]
  score  = rs * t                    (att_b dropped: softmax shift-invariant)
  e      = exp(score)
  praw[b,d] = sum_s (e[s]*rs[s]) * x[b,s,d]
  pooled = praw / sum_s e            (host; e shipped out raw)
  h      = pooled @ (ln_w[:,None]*geglu_w) + geglu_b;  out = val * gelu(gate)

Two NEFF launches (collective latency ~70us makes a fused NEFF slower):
  A) data-parallel pooling over batch (4 batches/core), x in bf16.
     Per x tile [128,1024]: ACT computes q via Square+accum_out (DVE takes
     QDVE of them for balance), DVE computes t via affine_mul_reduce,
     GpSimd runs the softmax smalls per half-batch chunk (the final chunk
     runs on DVE, which is idle by then), PE accumulates value matmuls in
     bf16, ACT evacuates PSUM.
  B) tensor-parallel GeGLU in bf16: host gathers pooled (128 KB), each core
     computes its 512 val+gate column pairs.
"""

import os
import numpy as np

B, S, D, OUT = 32, 2048, 1024, 4096
EPS = 1e-6
NCORES = 8
NB = B // NCORES          # batches per core
COLS = OUT // NCORES      # val columns per core
P = 128
NT = S // P               # seq tiles per batch
NC = 2                    # softmax chunks per batch
CT = NT // NC             # tiles per chunk
QDVE = 5                  # q-tiles offloaded ACT->DVE per core (balance)

_cache = {}


def _build_nc_pool():
    import concourse.bacc as bacc
    import concourse.mybir as mybir
    import concourse.tile as tile
    from contextlib import ExitStack

    f32 = mybir.dt.float32
    bf16 = mybir.dt.bfloat16
    AF = mybir.ActivationFunctionType
    OP = mybir.AluOpType

    nc = bacc.Bacc(
        "TRN2",
        target_bir_lowering=False,
        debug=False,
        enable_asserts=False,
        num_devices=NCORES,
    )

    x_d = nc.dram_tensor("x", [NB, S, D], bf16, kind="ExternalInput").ap()
    a_d = nc.dram_tensor("a", [1, D], bf16, kind="ExternalInput").ap()
    praw_d = nc.dram_tensor("praw", [NB, D], f32, kind="ExternalOutput").ap()
    e_d = nc.dram_tensor("e", [NB, P, NT], f32, kind="ExternalOutput").ap()

    # q-tiles computed on DVE instead of ACT, spread across the stream
    ntiles = NB * NT
    qdve_set = set()
    if QDVE:
        step = ntiles // QDVE
        qdve_set = {step // 2 + i * step for i in range(QDVE)}

    with tile.TileContext(nc) as tc, ExitStack() as ctx:
        singles = ctx.enter_context(tc.tile_pool(name="singles", bufs=1))
        xpool = ctx.enter_context(tc.tile_pool(name="xp", bufs=24))
        scr_a = ctx.enter_context(tc.tile_pool(name="scra", bufs=3))
        scr_v = ctx.enter_context(tc.tile_pool(name="scrv", bufs=3))
        small = ctx.enter_context(tc.tile_pool(name="small", bufs=4))
        psum_pool = ctx.enter_context(
            tc.tile_pool(name="pspool", bufs=2, space="PSUM")
        )

        a_bc = singles.tile([P, D], bf16)
        nc.sync.dma_start(out=a_bc, in_=a_d.to_broadcast([P, D]))

        for b in range(NB):
            pp = psum_pool.tile([1, D], f32, tag="acc")
            for c in range(NC):
                last_chunk = (b == NB - 1 and c == NC - 1)
                # smalls engine: GpSimd normally; DVE for the final chunk
                g = nc.vector if last_chunk else nc.gpsimd
                q_all = small.tile([P, CT], f32, tag="q")
                t_all = small.tile([P, CT], f32, tag="t")
                x_tiles = []
                for j in range(CT):
                    jj = c * CT + j
                    xt = xpool.tile([P, D], bf16, tag="x")
                    if b == 0 and c == 0 and j < 3:
                        # stripe the first tiles across 4 DMA queues so the
                        # engines start ~4us earlier (single-queue tile DMA
                        # takes ~5us)
                        for st in range(4):
                            nc.sync.dma_start(
                                out=xt[st * 32:(st + 1) * 32, :],
                                in_=x_d[b, jj * P + st * 32:
                                        jj * P + (st + 1) * 32, :])
                    else:
                        nc.sync.dma_start(
                            out=xt, in_=x_d[b, jj * P:(jj + 1) * P, :])
                    x_tiles.append(xt)
                    if b * NT + jj in qdve_set:
                        sq = scr_v.tile([P, D], bf16, tag="sqv")
                        nc.vector.affine_mul_reduce(
                            out=sq, accum_out=q_all[:, j:j + 1],
                            in0=xt, in1=xt, scale=1.0, bias=0.0)
                    else:
                        sq = scr_a.tile([P, D], bf16, tag="sqa")
                        nc.scalar.activation(out=sq, in_=xt, func=AF.Square,
                                             accum_out=q_all[:, j:j + 1])
                    tp = scr_v.tile([P, D], bf16, tag="tp")
                    nc.vector.affine_mul_reduce(
                        out=tp, accum_out=t_all[:, j:j + 1],
                        in0=xt, in1=a_bc, scale=1.0, bias=0.0)

                # softmax smalls: rs = (q/D+eps)^-1/2, 1 Newton step
                v = small.tile([P, CT], f32, tag="v")
                g.tensor_scalar(out=v, in0=q_all, scalar1=1.0 / D,
                                scalar2=EPS, op0=OP.mult, op1=OP.add)
                y = small.tile([P, CT], f32, tag="y")
                g.tensor_scalar(out=y, in0=v, scalar1=-0.5, scalar2=1.5,
                                op0=OP.mult, op1=OP.add)
                u = small.tile([P, CT], f32, tag="u")
                g.tensor_mul(u, y, y)
                g.tensor_mul(u, u, v)
                g.tensor_scalar(out=u, in0=u, scalar1=-0.5, scalar2=1.5,
                                op0=OP.mult, op1=OP.add)
                y1 = small.tile([P, CT], f32, tag="y1")
                g.tensor_mul(y1, y, u)
                sc = small.tile([P, CT], f32, tag="sc")
                g.tensor_mul(sc, t_all, y1)
                e_all = small.tile([P, CT], f32, tag="e")
                nc.scalar.activation(out=e_all, in_=sc, func=AF.Exp)
                c_all = small.tile([P, CT], bf16, tag="c")
                g.tensor_mul(c_all, e_all, y1)
                # host computes the softmax denominator from raw e
                nc.sync.dma_start(out=e_d[b, :, c * CT:(c + 1) * CT],
                                  in_=e_all)

                # value pass: praw[1,D] += c_j^T @ x_j  (bf16, fp32 PSUM)
                for j in range(CT):
                    for h in range(2):
                        nc.tensor.matmul(
                            pp[0:1, h * 512:(h + 1) * 512],
                            lhsT=c_all[:, j:j + 1],
                            rhs=x_tiles[j][:, h * 512:(h + 1) * 512],
                            start=(c == 0 and j == 0),
                            stop=(c == NC - 1 and j == CT - 1))
            pr_sb = small.tile([1, D], f32, tag="pr")
            nc.scalar.copy(pr_sb, pp)
            nc.sync.dma_start(out=praw_d[b:b + 1, :], in_=pr_sb)

    nc.compile()
    return nc


def _build_nc_geglu():
    import concourse.bacc as bacc
    import concourse.mybir as mybir
    import concourse.tile as tile
    from contextlib import ExitStack

    f32 = mybir.dt.float32
    bf16 = mybir.dt.bfloat16
    AF = mybir.ActivationFunctionType

    nc = bacc.Bacc(
        "TRN2",
        target_bir_lowering=False,
        debug=False,
        enable_asserts=False,
        num_devices=NCORES,
    )

    pT_d = nc.dram_tensor("pT", [P, 8, B], bf16, kind="ExternalInput").ap()
    w_d = nc.dram_tensor("w", [8, P, 2 * COLS], bf16, kind="ExternalInput").ap()
    bias_d = nc.dram_tensor("bias", [1, 2 * COLS], f32, kind="ExternalInput").ap()
    out_d = nc.dram_tensor("out", [B, COLS], f32, kind="ExternalOutput").ap()

    with tile.TileContext(nc) as tc, ExitStack() as ctx:
        singles = ctx.enter_context(tc.tile_pool(name="singles", bufs=1))
        tailp = ctx.enter_context(tc.tile_pool(name="tail", bufs=2))
        psum_pool = ctx.enter_context(
            tc.tile_pool(name="pspool", bufs=1, space="PSUM")
        )

        pT_sb = singles.tile([P, 8, B], bf16)
        nc.sync.dma_start(out=pT_sb, in_=pT_d)
        # half-chunk DMAs across queues so matmul k starts as soon as
        # its chunk lands
        w_sb = singles.tile([P, 8, 2 * COLS], bf16)
        for k in range(8):
            for hh in range(2):
                nc.sync.dma_start(
                    out=w_sb[:, k, hh * COLS:(hh + 1) * COLS],
                    in_=w_d[k][:, hh * COLS:(hh + 1) * COLS])
        bias_bc = singles.tile([B, 2 * COLS], f32)
        nc.sync.dma_start(out=bias_bc, in_=bias_d.to_broadcast([B, 2 * COLS]))

        hps = psum_pool.tile([B, 2 * COLS], f32, tag="acc")
        for k in range(8):
            for h in range(2):
                nc.tensor.matmul(
                    hps[:, h * COLS:(h + 1) * COLS],
                    lhsT=pT_sb[:, k, :],
                    rhs=w_sb[:, k, h * COLS:(h + 1) * COLS],
                    start=(k == 0), stop=(k == 7))
        hv = tailp.tile([B, COLS], f32, tag="hv")
        nc.vector.tensor_add(hv, hps[:, 0:COLS], bias_bc[:, 0:COLS])
        hg = tailp.tile([B, COLS], f32, tag="hg")
        nc.vector.tensor_add(hg, hps[:, COLS:2 * COLS], bias_bc[:, COLS:2 * COLS])
        gg = tailp.tile([B, COLS], f32, tag="gg")
        nc.scalar.activation(out=gg, in_=hg, func=AF.Gelu)
        outt = tailp.tile([B, COLS], f32, tag="outt")
        nc.vector.tensor_mul(outt, hv, gg)
        nc.sync.dma_start(out=out_d, in_=outt)

    nc.compile()
    return nc


def _pool_in_maps(x, ln_w, att_w):
    import ml_dtypes
    bf = ml_dtypes.bfloat16
    a = (ln_w * att_w[:, 0]).astype(bf).reshape(1, D)
    xc = np.ascontiguousarray(x.astype(bf))
    return [
        {"x": xc[r * NB:(r + 1) * NB], "a": a}
        for r in range(NCORES)
    ]


def _geglu_in_maps(pooled_full, ln_w, geglu_w, geglu_b):
    import ml_dtypes
    bf = ml_dtypes.bfloat16
    pT = np.ascontiguousarray(
        pooled_full.T.astype(bf).reshape(8, P, B).transpose(1, 0, 2))
    Wf = ln_w[:, None] * geglu_w
    maps = []
    for r in range(NCORES):
        vs = slice(r * COLS, (r + 1) * COLS)
        gs = slice(OUT + r * COLS, OUT + (r + 1) * COLS)
        wr = np.ascontiguousarray(
            np.concatenate([Wf[:, vs], Wf[:, gs]], axis=1)
            .astype(bf).reshape(8, P, 2 * COLS))
        br = np.ascontiguousarray(
            np.concatenate([geglu_b[vs], geglu_b[gs]])
        ).reshape(1, 2 * COLS).astype(np.float32)
        maps.append({"pT": pT, "w": wr, "bias": br})
    return maps


LAST_RESULTS = None


def kernel(x, ln_w, att_w, att_b, geglu_w, geglu_b):
    global LAST_RESULTS
    from concourse.bass_utils import run_bass_kernel_spmd

    x = np.asarray(x, dtype=np.float32)
    ln_w = np.asarray(ln_w, dtype=np.float32)
    att_w = np.asarray(att_w, dtype=np.float32)
    geglu_w = np.asarray(geglu_w, dtype=np.float32)
    geglu_b = np.asarray(geglu_b, dtype=np.float32)
    # att_b is mathematically irrelevant (softmax shift-invariance)

    if "A" not in _cache:
        _cache["A"] = _build_nc_pool()
    if "B" not in _cache:
        _cache["B"] = _build_nc_geglu()

    trace = os.environ.get("KERNEL_TRACE", "0") == "1"

    res_a = run_bass_kernel_spmd(
        _cache["A"], _pool_in_maps(x, ln_w, att_w),
        core_ids=list(range(NCORES)), trace=trace,
    )
    praw = np.concatenate(
        [res_a.results[r]["praw"] for r in range(NCORES)], axis=0
    ).astype(np.float64)
    esum = np.stack(
        [res_a.results[r]["e"].astype(np.float64).sum(axis=(1, 2))
         for r in range(NCORES)]
    ).reshape(B)
    pooled_full = (praw / esum[:, None]).astype(np.float32)

    res_b = run_bass_kernel_spmd(
        _cache["B"], _geglu_in_maps(pooled_full, ln_w, geglu_w, geglu_b),
        core_ids=list(range(NCORES)), trace=trace,
    )
    LAST_RESULTS = (res_a, res_b)
    out = np.concatenate(
        [res_b.results[r]["out"] for r in range(NCORES)], axis=1
    )
    return out.astype(np.float32)


# revision 9
# speedup vs baseline: 2.7269x; 1.2204x over previous
"""Trainium2 Bass kernel for AttentionWithGeGLU pooling.

Math (per batch row b):
  q[s]   = sum_d x[b,s,d]^2
  rs[s]  = (q/D + eps)^-1/2          (1 Newton step from y0 = 1.5 - 0.5 v)
  t[s]   = sum_d x[b,s,d] * (ln_w*att_w)[d]
  score  = rs * t                    (att_b dropped: softmax shift-invariant)
  e      = exp(score)
  praw[b,d] = sum_s (e[s]*rs[s]) * x[b,s,d]
  pooled = praw / sum_s e            (host; e shipped out raw)
  h      = pooled @ (ln_w[:,None]*geglu_w) + geglu_b;  out = val * gelu(gate)

Two NEFF launches (collective latency ~70us makes a fused NEFF slower).

Pool NEFF engine split per x tile [128,1024]: ACT computes q via
Square+accum_out (DVE takes QDVE of them for balance), DVE computes t via
affine_mul_reduce, GpSimd runs the softmax smalls, PE accumulates value
matmuls in bf16, ACT evacuates PSUM.  The exp/c/value-matmul/evac chain of
each half-batch chunk is EMITTED one chunk late: engines execute their
queues in order, so an instruction whose cross-engine deps aren't met yet
(exp waits on GpSimd, evac waits on PE) head-of-line-blocks everything
behind it.  Deferring the emission point keeps ACT/DVE streaming.  The
final chunk's smalls run on DVE (idle by then) to shorten the tail.
"""

import os
import numpy as np

B, S, D, OUT = 32, 2048, 1024, 4096
EPS = 1e-6
NCORES = 8
NB = B // NCORES          # batches per core
COLS = OUT // NCORES      # val columns per core
P = 128
NT = S // P               # seq tiles per batch
NC = 2                    # softmax chunks per batch
CT = NT // NC             # tiles per chunk
QDVE = 4                  # q-tiles offloaded ACT->DVE (balance), not in last chunk
STRIPE0 = 6               # first tiles striped across 4 DMA queues

_cache = {}


def _build_nc_pool():
    import concourse.bacc as bacc
    import concourse.mybir as mybir
    import concourse.tile as tile
    from contextlib import ExitStack

    f32 = mybir.dt.float32
    bf16 = mybir.dt.bfloat16
    AF = mybir.ActivationFunctionType
    OP = mybir.AluOpType

    nc = bacc.Bacc(
        "TRN2",
        target_bir_lowering=False,
        debug=False,
        enable_asserts=False,
        num_devices=NCORES,
    )

    x_d = nc.dram_tensor("x", [NB, S, D], bf16, kind="ExternalInput").ap()
    a_d = nc.dram_tensor("a", [1, D], bf16, kind="ExternalInput").ap()
    praw_d = nc.dram_tensor("praw", [NB, D], f32, kind="ExternalOutput").ap()
    e_d = nc.dram_tensor("e", [NB, P, NT], f32, kind="ExternalOutput").ap()

    ntiles = NB * NT
    qdve_set = set()
    if QDVE:
        step = (ntiles - CT) // QDVE
        qdve_set = {step // 2 + i * step for i in range(QDVE)}

    NCH = NB * NC           # total chunks

    with tile.TileContext(nc) as tc, ExitStack() as ctx:
        singles = ctx.enter_context(tc.tile_pool(name="singles", bufs=1))
        xpool = ctx.enter_context(tc.tile_pool(name="xp", bufs=28))
        scr_a = ctx.enter_context(tc.tile_pool(name="scra", bufs=3))
        scr_v = ctx.enter_context(tc.tile_pool(name="scrv", bufs=3))
        small = ctx.enter_context(tc.tile_pool(name="small", bufs=4))
        psum_pool = ctx.enter_context(
            tc.tile_pool(name="pspool", bufs=3, space="PSUM")
        )

        a_bc = singles.tile([P, D], bf16)
        nc.sync.dma_start(out=a_bc, in_=a_d.to_broadcast([P, D]))

        pp_of = {}          # batch -> psum tile
        e_of = {}           # batch -> e_all tile
        chunk_info = {}     # g -> dict(tiles, y1, sc, ...)

        def emit_block(g):
            """exp, c, value matmuls (+ e DMA at batch end) for chunk g."""
            b, c = divmod(g, NC)
            info = chunk_info.pop(g)
            gv = info["g_eng"]
            if b not in e_of:
                e_of[b] = small.tile([P, NT], f32, tag="e", name=f"eall{b}")
            e_all = e_of[b]
            esl = e_all[:, c * CT:(c + 1) * CT]
            nc.scalar.activation(out=esl, in_=info["sc"], func=AF.Exp)
            c_all = small.tile([P, CT], bf16, tag="c")
            gv.tensor_mul(c_all, esl, info["y1"])
            if c == NC - 1:
                nc.sync.dma_start(out=e_d[b], in_=e_all)
                e_of.pop(b)
            if b not in pp_of:
                pp_of[b] = psum_pool.tile([1, D], f32, tag="acc", name=f"pp{b}")
            pp = pp_of[b]
            for j in range(CT):
                for h in range(2):
                    nc.tensor.matmul(
                        pp[0:1, h * 512:(h + 1) * 512],
                        lhsT=c_all[:, j:j + 1],
                        rhs=info["tiles"][j][:, h * 512:(h + 1) * 512],
                        start=(c == 0 and j == 0),
                        stop=(c == NC - 1 and j == CT - 1))

        def emit_evac(b):
            pp = pp_of.pop(b)
            pr_sb = small.tile([1, D], f32, tag="pr")
            nc.scalar.copy(pr_sb, pp)
            nc.sync.dma_start(out=praw_d[b:b + 1, :], in_=pr_sb)

        for g in range(NCH):
            b, c = divmod(g, NC)
            last = (g == NCH - 1)
            # ---- step 1: stream this chunk's tiles; q on ACT, t on DVE ----
            x_tiles = []
            q_all = small.tile([P, CT], f32, tag="q")
            t_all = small.tile([P, CT], f32, tag="t")
            for j in range(CT):
                jj = c * CT + j
                gt = b * NT + jj
                xt = xpool.tile([P, D], bf16, tag="x")
                if gt < STRIPE0:
                    for st in range(4):
                        nc.sync.dma_start(
                            out=xt[st * 32:(st + 1) * 32, :],
                            in_=x_d[b, jj * P + st * 32:
                                    jj * P + (st + 1) * 32, :])
                else:
                    nc.sync.dma_start(
                        out=xt, in_=x_d[b, jj * P:(jj + 1) * P, :])
                x_tiles.append(xt)
                if gt in qdve_set:
                    sq = scr_v.tile([P, D], bf16, tag="sqv")
                    nc.vector.affine_mul_reduce(
                        out=sq, accum_out=q_all[:, j:j + 1],
                        in0=xt, in1=xt, scale=1.0, bias=0.0)
                else:
                    sq = scr_a.tile([P, D], bf16, tag="sqa")
                    nc.scalar.activation(out=sq, in_=xt, func=AF.Square,
                                         accum_out=q_all[:, j:j + 1])
                tp = scr_v.tile([P, D], bf16, tag="tp")
                nc.vector.affine_mul_reduce(
                    out=tp, accum_out=t_all[:, j:j + 1],
                    in0=xt, in1=a_bc, scale=1.0, bias=0.0)

            # ---- step 2: deferred work whose deps are met by now ----
            if g >= 1:
                emit_block(g - 1)
            if g >= 3 and g % NC == 1:      # chunk (b,1): evac batch b-1
                emit_evac(b - 1)

            # ---- step 3: softmax smalls (GpSimd; DVE for the last chunk) --
            gv = nc.vector if last else nc.gpsimd
            v = small.tile([P, CT], f32, tag="v")
            gv.tensor_scalar(out=v, in0=q_all, scalar1=1.0 / D,
                             scalar2=EPS, op0=OP.mult, op1=OP.add)
            y = small.tile([P, CT], f32, tag="y")
            gv.tensor_scalar(out=y, in0=v, scalar1=-0.5, scalar2=1.5,
                             op0=OP.mult, op1=OP.add)
            u = small.tile([P, CT], f32, tag="u")
            gv.tensor_mul(u, y, y)
            gv.tensor_mul(u, u, v)
            gv.tensor_scalar(out=u, in0=u, scalar1=-0.5, scalar2=1.5,
                             op0=OP.mult, op1=OP.add)
            y1 = small.tile([P, CT], f32, tag="y1")
            gv.tensor_mul(y1, y, u)
            sc = small.tile([P, CT], f32, tag="sc")
            gv.tensor_mul(sc, t_all, y1)
            chunk_info[g] = {"tiles": x_tiles, "y1": y1, "sc": sc,
                             "g_eng": gv}

        # ---- tail: final chunk's block + remaining evac ----
        emit_block(NCH - 1)
        emit_evac(NB - 1)

    nc.compile()
    return nc


def _build_nc_geglu():
    import concourse.bacc as bacc
    import concourse.mybir as mybir
    import concourse.tile as tile
    from contextlib import ExitStack

    f32 = mybir.dt.float32
    bf16 = mybir.dt.bfloat16
    AF = mybir.ActivationFunctionType

    nc = bacc.Bacc(
        "TRN2",
        target_bir_lowering=False,
        debug=False,
        enable_asserts=False,
        num_devices=NCORES,
    )

    pT_d = nc.dram_tensor("pT", [P, 8, B], bf16, kind="ExternalInput").ap()
    w_d = nc.dram_tensor("w", [8, P, 2 * COLS], bf16, kind="ExternalInput").ap()
    bias_d = nc.dram_tensor("bias", [1, 2 * COLS], f32, kind="ExternalInput").ap()
    out_d = nc.dram_tensor("out", [B, COLS], f32, kind="ExternalOutput").ap()

    with tile.TileContext(nc) as tc, ExitStack() as ctx:
        singles = ctx.enter_context(tc.tile_pool(name="singles", bufs=1))
        tailp = ctx.enter_context(tc.tile_pool(name="tail", bufs=2))
        psum_pool = ctx.enter_context(
            tc.tile_pool(name="pspool", bufs=1, space="PSUM")
        )

        pT_sb = singles.tile([P, 8, B], bf16)
        nc.sync.dma_start(out=pT_sb, in_=pT_d)
        # per-chunk DMAs so matmul k starts as soon as chunk k lands
        w_sb = singles.tile([P, 8, 2 * COLS], bf16)
        for k in range(8):
            nc.sync.dma_start(out=w_sb[:, k], in_=w_d[k])
        bias_bc = singles.tile([B, 2 * COLS], f32)
        nc.sync.dma_start(out=bias_bc, in_=bias_d.to_broadcast([B, 2 * COLS]))

        hps = psum_pool.tile([B, 2 * COLS], f32, tag="acc")
        for k in range(8):
            for h in range(2):
                nc.tensor.matmul(
                    hps[:, h * COLS:(h + 1) * COLS],
                    lhsT=pT_sb[:, k, :],
                    rhs=w_sb[:, k, h * COLS:(h + 1) * COLS],
                    start=(k == 0), stop=(k == 7))
        # gate half first so ACT's gelu overlaps the value-half add on DVE
        hg = tailp.tile([B, COLS], f32, tag="hg")
        nc.vector.tensor_add(hg, hps[:, COLS:2 * COLS], bias_bc[:, COLS:2 * COLS])
        gg = tailp.tile([B, COLS], f32, tag="gg")
        nc.scalar.activation(out=gg, in_=hg, func=AF.Gelu)
        hv = tailp.tile([B, COLS], f32, tag="hv")
        nc.vector.tensor_add(hv, hps[:, 0:COLS], bias_bc[:, 0:COLS])
        outt = tailp.tile([B, COLS], f32, tag="outt")
        nc.vector.tensor_mul(outt, hv, gg)
        nc.sync.dma_start(out=out_d, in_=outt)

    nc.compile()
    return nc


def _pool_in_maps(x, ln_w, att_w):
    import ml_dtypes
    bf = ml_dtypes.bfloat16
    a = (ln_w * att_w[:, 0]).astype(bf).reshape(1, D)
    xc = np.ascontiguousarray(x.astype(bf))
    return [
        {"x": xc[r * NB:(r + 1) * NB], "a": a}
        for r in range(NCORES)
    ]


def _geglu_in_maps(pooled_full, ln_w, geglu_w, geglu_b):
    import ml_dtypes
    bf = ml_dtypes.bfloat16
    pT = np.ascontiguousarray(
        pooled_full.T.astype(bf).reshape(8, P, B).transpose(1, 0, 2))
    Wf = ln_w[:, None] * geglu_w
    maps = []
    for r in range(NCORES):
        vs = slice(r * COLS, (r + 1) * COLS)
        gs = slice(OUT + r * COLS, OUT + (r + 1) * COLS)
        wr = np.ascontiguousarray(
            np.concatenate([Wf[:, vs], Wf[:, gs]], axis=1)
            .astype(bf).reshape(8, P, 2 * COLS))
        br = np.ascontiguousarray(
            np.concatenate([geglu_b[vs], geglu_b[gs]])
        ).reshape(1, 2 * COLS).astype(np.float32)
        maps.append({"pT": pT, "w": wr, "bias": br})
    return maps


LAST_RESULTS = None


def kernel(x, ln_w, att_w, att_b, geglu_w, geglu_b):
    global LAST_RESULTS
    from concourse.bass_utils import run_bass_kernel_spmd

    x = np.asarray(x, dtype=np.float32)
    ln_w = np.asarray(ln_w, dtype=np.float32)
    att_w = np.asarray(att_w, dtype=np.float32)
    geglu_w = np.asarray(geglu_w, dtype=np.float32)
    geglu_b = np.asarray(geglu_b, dtype=np.float32)
    # att_b is mathematically irrelevant (softmax shift-invariance)

    if "A" not in _cache:
        _cache["A"] = _build_nc_pool()
    if "B" not in _cache:
        _cache["B"] = _build_nc_geglu()

    trace = os.environ.get("KERNEL_TRACE", "0") == "1"

    res_a = run_bass_kernel_spmd(
        _cache["A"], _pool_in_maps(x, ln_w, att_w),
        core_ids=list(range(NCORES)), trace=trace,
    )
    praw = np.concatenate(
        [res_a.results[r]["praw"] for r in range(NCORES)], axis=0
    ).astype(np.float64)
    esum = np.stack(
        [res_a.results[r]["e"].astype(np.float64).sum(axis=(1, 2))
         for r in range(NCORES)]
    ).reshape(B)
    pooled_full = (praw / esum[:, None]).astype(np.float32)

    res_b = run_bass_kernel_spmd(
        _cache["B"], _geglu_in_maps(pooled_full, ln_w, geglu_w, geglu_b),
        core_ids=list(range(NCORES)), trace=trace,
    )
    LAST_RESULTS = (res_a, res_b)
    out = np.concatenate(
        [res_b.results[r]["out"] for r in range(NCORES)], axis=1
    )
    return out.astype(np.float32)
